# revision 2
# baseline (speedup 1.0000x reference)
"""Blockwise butterfly rotation (nn_BlockwiseButterflyRotation) - TRN2 Bass kernel.

Full inputs: x (4, 4096, 4096) f32, angles (16, 8, 128) f32.
Math: x is split into 16 independent 256-wide blocks; each block's rows are
rotated by an 8-stage butterfly. The composed per-block rotation is a dense
256x256 matrix C_b = B_b^T, so out = x @ blockdiag(C). The kernel builds C
on-device from the angles and runs the bulk work as PE matmuls.

Sharding: data-parallel over rows - x.reshape(16384, 4096) split into 8
contiguous shards of 2048 rows; angles (gathered into per-partition coeff
layout, pure indexing) replicated to all cores.

Per-core dataflow, per 128-row tile:
  DMA in [128, 4096] f32
  -> 32x PE transpose (f32, exact) of 128x128 chunks -> PSUM
  -> PSUM->SBUF copy, rounding to float32r (DVE/ACT alternating)
  -> 32x PE matmul f32r: out[128, 256] += xT_chunk^T @ C_chunk (N=256 ->
     full PE rate for f32r)
  -> PSUM->SBUF copy f32 -> DMA out

C build (once, overlapped with first loads): two-level butterfly
factorization C[16g+u, 16w+v] = LT_g[u,v] * HT_v[g,w]; LT (stages 0-3) and
HT (stages 4-7) are built by applying 16x16 butterflies to identity patterns
with free-dim-only pairing on the DVE; cos/sin via ScalarE Sin (cos =
sin(x + pi/2)); HT's u-replication via 16 selector matmuls on the PE;
final combine is 32 strided tensor_tensor multiplies.
"""
import math
import os

import numpy as np

from concourse import bacc, mybir, tile, masks
from concourse.bass_utils import run_bass_kernel_spmd

F32 = mybir.dt.float32
F32R = mybir.dt.float32r

DIM = 4096
NB = 16
BLOCK = 256
HALF_PI = math.pi / 2.0

N_CORES = 8
R_TOTAL = 4 * 4096
R_CORE = R_TOTAL // N_CORES  # 2048

LAST_RESULT = None  # BassKernelResults of the most recent kernel() call
_NC_CACHE = {}


def gather_angles(angles: np.ndarray) -> np.ndarray:
    """angles [16, 8, 128] f32 -> ang [128, 1536] f32 (angL 4x256 | angH 4x128).

    Pure gather (indexing only, no arithmetic) into the per-partition
    coefficient layouts the kernel's butterfly-stage APs iterate.
    """
    angles = np.asarray(angles)
    assert angles.shape == (NB, 8, 128)
    out = np.empty((128, 1536), dtype=np.float32)
    for s in range(4):
        sig = 1 << s
        col = np.empty((128, 256), dtype=np.float32)
        for g0 in range(8):
            row = np.empty((16, 2, 8), dtype=np.float32)
            for kc in range(2):
                g = 8 * kc + g0
                for vg in range(8 // sig):
                    for t in range(sig):
                        row[:, kc, vg * sig + t] = angles[:, s, 8 * g + vg * sig + t]
            col[16 * g0:16 * g0 + 16, :] = row.reshape(1, 256)
        out[:, 256 * s:256 * (s + 1)] = col
    for sp in range(4):
        sigp = 1 << sp
        col = np.empty((128, 128), dtype=np.float32)
        for b in range(16):
            row = np.empty((16, 8), dtype=np.float32)
            for v in range(16):
                for wg in range(8 // sigp):
                    for t in range(sigp):
                        row[v, wg * sigp + t] = angles[b, sp + 4, wg * 16 * sigp + 16 * t + v]
            col[b::16, :] = row.reshape(1, 128)
        out[:, 1024 + 128 * sp:1024 + 128 * (sp + 1)] = col
    return out


def _butterfly_stage(nc, pool, data, n1, n2, sig, cos_ap, sin_ap):
    """One butterfly stage on `data` viewed as [p, n1, n2, ng, 2, sig];
    pairs along the (ng, 2, sig) axis group. cos/sin APs iterate
    [p, n1, n2, ng, sig]."""
    ng = 8 // sig
    v = data.rearrange("p (n1 n2 vg h t) -> p n1 n2 vg h t",
                       n1=n1, n2=n2, vg=ng, h=2, t=sig)
    a = v[:, :, :, :, 0, :]
    b_ = v[:, :, :, :, 1, :]
    half = n1 * n2 * 8
    t1 = pool.tile([128, half], F32, name="bt_t1", tag="bt_t1")
    t2 = pool.tile([128, half], F32, name="bt_t2", tag="bt_t2")
    t3 = pool.tile([128, half], F32, name="bt_t3", tag="bt_t3")
    t4 = pool.tile([128, half], F32, name="bt_t4", tag="bt_t4")
    tv = lambda t: t[:].rearrange("p (n1 n2 vg t) -> p n1 n2 vg t",
                                  n1=n1, n2=n2, vg=ng, t=sig)
    nc.vector.tensor_mul(tv(t1), a, cos_ap)
    nc.vector.tensor_mul(tv(t2), b_, sin_ap)
    nc.vector.tensor_mul(tv(t3), a, sin_ap)
    nc.vector.tensor_mul(tv(t4), b_, cos_ap)
    nc.vector.tensor_sub(a, tv(t1), tv(t2))
    nc.vector.tensor_add(b_, tv(t3), tv(t4))


def build_nc(R: int, repeat: int | None = None):
    """repeat: if set, wrap the main loop in an on-device For_i that re-runs
    it `repeat` times on the same data (identical output; used by the timing
    harness to resolve per-pass time above the dispatch noise floor)."""
    assert R % 128 == 0
    RT = R // 128
    nc = bacc.Bacc("TRN2", target_bir_lowering=False, debug=False)

    X = nc.dram_tensor("x", [R, DIM], F32, kind="ExternalInput").ap()
    ANG = nc.dram_tensor("ang", [128, 1536], F32, kind="ExternalInput").ap()
    OUT = nc.dram_tensor("out", [R, DIM], F32, kind="ExternalOutput").ap()

    with tile.TileContext(nc) as tc:
        with tc.tile_pool(name="const", bufs=1) as cpool:
            ident = cpool.tile([128, 128], F32)
            masks.make_identity(nc, ident[:])
            halfpi = cpool.tile([128, 1], F32)
            nc.gpsimd.memset(halfpi[:], HALF_PI)
            CT = cpool.tile([128, 8192], F32R)  # C: [p=k%128, (b, kc, w, v)]

            # ---------------- C build ----------------
            with tc.tile_pool(name="build", bufs=1) as bpool:
                angsb = bpool.tile([128, 1536], F32)
                nc.sync.dma_start(out=angsb[:], in_=ANG)

                # LS [p=(g0,u), (b:16, kc:2, v:16)] init = delta(v == p mod 16).
                # Compute APs must start at partition 0/32/64/96, so use
                # full-tile passes: pass k fills where p - v - 16k == 0.
                LS = bpool.tile([128, 512], F32)
                nc.gpsimd.memset(LS[:], 0.0)
                lsv = LS[:].rearrange("p (b kc v) -> p b kc v", b=16, kc=2, v=16)
                for k in range(8):
                    nc.gpsimd.affine_select(
                        out=lsv, in_=lsv,
                        pattern=[[0, 16], [0, 2], [-1, 16]],
                        compare_op=mybir.AluOpType.not_equal,
                        fill=1.0, base=-16 * k, channel_multiplier=1)

                # HSB [p=(g0,b), (kc:2, v:16, w:16)] init = delta(w == 8*kc+g0).
                # Pass b fills where 16w - 128kc - p + b == 0.
                HSB = bpool.tile([128, 512], F32)
                nc.gpsimd.memset(HSB[:], 0.0)
                hsv = HSB[:].rearrange("p (kc v w) -> p kc v w", kc=2, v=16, w=16)
                for b in range(16):
                    nc.gpsimd.affine_select(
                        out=hsv, in_=hsv,
                        pattern=[[-128, 2], [0, 16], [16, 16]],
                        compare_op=mybir.AluOpType.not_equal,
                        fill=1.0, base=b, channel_multiplier=-1)

                # LS stages 0-3
                for s in range(4):
                    sig = 1 << s
                    ng = 8 // sig
                    cosT = bpool.tile([128, 256], F32, name="cosL", tag="cosL")
                    sinT = bpool.tile([128, 256], F32, name="sinL", tag="sinL")
                    asl = angsb[:, 256 * s:256 * (s + 1)]
                    nc.scalar.activation(cosT[:], asl, mybir.ActivationFunctionType.Sin,
                                         bias=halfpi[:], scale=1.0)
                    nc.scalar.activation(sinT[:], asl, mybir.ActivationFunctionType.Sin,
                                         bias=0.0, scale=1.0)
                    cv = cosT[:].rearrange("p (b kc vg t) -> p b kc vg t",
                                           b=16, kc=2, vg=ng, t=sig)
                    sv = sinT[:].rearrange("p (b kc vg t) -> p b kc vg t",
                                           b=16, kc=2, vg=ng, t=sig)
                    _butterfly_stage(nc, bpool, LS[:], 16, 2, sig, cv, sv)

                # HSB stages 4-7 (coeffs independent of kc -> broadcast)
                for sp in range(4):
                    sigp = 1 << sp
                    ng = 8 // sigp
                    cosT = bpool.tile([128, 128], F32, name="cosH", tag="cosH")
                    sinT = bpool.tile([128, 128], F32, name="sinH", tag="sinH")
                    asl = angsb[:, 1024 + 128 * sp:1024 + 128 * (sp + 1)]
                    nc.scalar.activation(cosT[:], asl, mybir.ActivationFunctionType.Sin,
                                         bias=halfpi[:], scale=1.0)
                    nc.scalar.activation(sinT[:], asl, mybir.ActivationFunctionType.Sin,
                                         bias=0.0, scale=1.0)
                    cv = cosT[:].rearrange("p (v vg t) -> p v vg t", v=16, vg=ng, t=sigp) \
                        .unsqueeze(1).to_broadcast((128, 2, 16, ng, sigp))
                    sv = sinT[:].rearrange("p (v vg t) -> p v vg t", v=16, vg=ng, t=sigp) \
                        .unsqueeze(1).to_broadcast((128, 2, 16, ng, sigp))
                    _butterfly_stage(nc, bpool, HSB[:], 2, 16, sigp, cv, sv)

                # HS[16g0+u, b*512+(kc,v,w)] = HSB[16g0+b, (kc,v,w)] via 16
                # selector matmuls: W_b^T @ HSB broadcasts row b of each
                # 16-partition group to all 16 u-lanes.
                HS = bpool.tile([128, 8192], F32)
                with tc.tile_pool(name="psR", bufs=2, space="PSUM") as psR:
                    for b in range(16):
                        Wb = bpool.tile([128, 128], F32, name="Wb", tag="Wb")
                        nc.gpsimd.memset(Wb[:], 0.0)
                        wv = Wb[:].rearrange("p (mg mu) -> p mg mu", mg=8, mu=16)
                        nc.gpsimd.affine_select(
                            out=wv, in_=wv,
                            pattern=[[-16, 8], [0, 16]],
                            compare_op=mybir.AluOpType.not_equal,
                            fill=1.0, base=-b, channel_multiplier=1)
                        psr = psR.tile([128, 512], F32, name="psr", tag="psr")
                        nc.tensor.matmul(psr[:], Wb[:], HSB[:], start=True, stop=True)
                        if b % 2 == 0:
                            nc.vector.tensor_copy(HS[:, 512 * b:512 * (b + 1)], psr[:])
                        else:
                            nc.scalar.copy(HS[:, 512 * b:512 * (b + 1)], psr[:])

                # Combine: CTf[p, b, kc, w, v] = LS[p, b, kc, v] * HS[p, b, kc, v, w]
                CTf = bpool.tile([128, 8192], F32)
                for b in range(16):
                    for kc in range(2):
                        o = CTf[:, (b * 2 + kc) * 256:(b * 2 + kc) * 256 + 256] \
                            .rearrange("p (w v) -> p w v", w=16, v=16)
                        i0 = LS[:, (b * 32 + kc * 16):(b * 32 + kc * 16) + 16] \
                            .unsqueeze(1).to_broadcast((128, 16, 16))
                        i1 = HS[:, (b * 512 + kc * 256):(b * 512 + kc * 256) + 256] \
                            .rearrange("p (v w) -> p w v", v=16, w=16)
                        nc.vector.tensor_mul(o, i0, i1)
                nc.vector.tensor_copy(CT[:], CTf[:])  # round once to f32r

            # ---------------- main loop ----------------
            import contextlib
            with tc.tile_pool(name="xin", bufs=3) as xpool, \
                 tc.tile_pool(name="xt", bufs=2) as xtpool, \
                 tc.tile_pool(name="outp", bufs=2) as opool, \
                 tc.tile_pool(name="psT", bufs=3, space="PSUM") as psT, \
                 tc.tile_pool(name="psO", bufs=3, space="PSUM") as psO, \
                 (tc.For_i(0, repeat, 1) if repeat else contextlib.nullcontext()):
                for r in range(RT):
                    xin = xpool.tile([128, DIM], F32, name="xin", tag="xin")
                    nc.sync.dma_start(out=xin[:], in_=X[r * 128:(r + 1) * 128, :])

                    xT = xtpool.tile([128, DIM], F32R, name="xT", tag="xT")
                    for j in range(8):
                        pst = psT.tile([128, 512], F32, name="pst", tag="pst")
                        for q in range(4):
                            i = 4 * j + q
                            nc.tensor.transpose(
                                pst[:, 128 * q:128 * (q + 1)],
                                xin[:, 128 * i:128 * (i + 1)], ident[:])
                        if j % 2 == 0:
                            nc.vector.tensor_copy(xT[:, 512 * j:512 * (j + 1)], pst[:])
                        else:
                            nc.scalar.copy(xT[:, 512 * j:512 * (j + 1)], pst[:])

                    outt = opool.tile([128, DIM], F32, name="outt", tag="outt")
                    for jb in range(8):
                        pso = psO.tile([128, 512], F32, name="pso", tag="pso")
                        for q in range(2):
                            b = 2 * jb + q
                            for kc in range(2):
                                i = 2 * b + kc
                                nc.tensor.matmul(
                                    pso[:, 256 * q:256 * (q + 1)],
                                    xT[:, 128 * i:128 * (i + 1)],
                                    CT[:, 256 * i:256 * (i + 1)],
                                    start=(kc == 0), stop=(kc == 1))
                        if jb % 2 == 0:
                            nc.vector.tensor_copy(outt[:, 512 * jb:512 * (jb + 1)], pso[:])
                        else:
                            nc.scalar.copy(outt[:, 512 * jb:512 * (jb + 1)], pso[:])
                    nc.gpsimd.dma_start(out=OUT[r * 128:(r + 1) * 128, :], in_=outt[:])

    nc.compile()
    return nc


def _get_nc():
    if "nc" not in _NC_CACHE:
        _NC_CACHE["nc"] = build_nc(R_CORE)
    return _NC_CACHE["nc"]


def kernel(x: np.ndarray, angles: np.ndarray) -> np.ndarray:
    global LAST_RESULT
    x = np.asarray(x)
    angles = np.asarray(angles)
    orig_shape = x.shape
    xf = np.ascontiguousarray(x.reshape(R_TOTAL, DIM), dtype=np.float32)
    ang = gather_angles(angles.astype(np.float32, copy=False))

    nc = _get_nc()
    in_maps = [
        {"x": np.ascontiguousarray(xf[c * R_CORE:(c + 1) * R_CORE]), "ang": ang}
        for c in range(N_CORES)
    ]
    trace = os.environ.get("BFK_TRACE", "") == "1"
    res = run_bass_kernel_spmd(nc, in_maps, list(range(N_CORES)), trace=trace)
    LAST_RESULT = res
    out = np.concatenate([res.results[c]["out"] for c in range(N_CORES)], axis=0)
    return out.reshape(orig_shape).astype(x.dtype, copy=False)
